# revision 1
# baseline (speedup 1.0000x reference)
"""Trainium2 Bass kernel for nn_AttentionHead (softmax over query axis).

Sharding: 8 cores = 4 batches x 2 halves. Core c handles batch c//2 and
row-parity h=c%2: local 128-row chunk lc <-> global chunk g=2*lc+h.
Per core:
  - cast x rows + weights to bf16 during DMA
  - PE-transpose x tiles -> xT [E-chunk, 1024 t]
  - projections qT/kT [128 D, 1024 t] and vT -> v natural [t, D]
  - AllGather kT, v across the pair (replica groups of 2)
  - scores sT[s, t] = kT_blk.T @ qT, exp (scale 1/sqrt(128)) with per-key
    column sums (softmax normalizer is over the QUERY axis), causal mask
    via host-supplied mask tiles (h=0: [tri, zeros], h=1: [ones, tri])
  - AllReduce the [128,16] normalizer partials across the pair
  - z[t, :] = sum_s E[s,t] * (v[s,:]/Z[s])
Host assembles the 8 core outputs back into [4, 2048, 128].
"""
import sys

for _p in ("/opt/trn_rl_repo",):
    if _p not in sys.path:
        sys.path.append(_p)

import numpy as np
import ml_dtypes

import concourse.bass as bass
import concourse.mybir as mybir
import concourse.tile as tile
from concourse import bacc
from concourse.bass import ds, ts
from concourse.bass_utils import run_bass_kernel_spmd
from concourse.masks import make_identity

BF16 = mybir.dt.bfloat16
F32 = mybir.dt.float32
AF = mybir.ActivationFunctionType
ALU = mybir.AluOpType
AX = mybir.AxisListType

B, T, E, D = 4, 2048, 2048, 128
NLC = 8          # local 128-row chunks per core
NE = 16          # E chunks of 128
NSB = 16         # key blocks of 128
SCALE = 1.0 / np.sqrt(D)
N_CORES = 8
REPLICA_GROUPS = [[0, 1], [2, 3], [4, 5], [6, 7]]


def gpos(g: int) -> int:
    """Global 128-chunk index -> position in the pair-gathered buffer."""
    return (g % 2) * 8 + g // 2


def build_nc():
    nc = bacc.Bacc("TRN2", target_bir_lowering=False, debug=False,
                   num_devices=N_CORES)
    x = nc.dram_tensor("x", [NLC * 128, E], F32, kind="ExternalInput")
    wq = nc.dram_tensor("wq", [E, D], F32, kind="ExternalInput")
    wk = nc.dram_tensor("wk", [E, D], F32, kind="ExternalInput")
    wv = nc.dram_tensor("wv", [E, D], F32, kind="ExternalInput")
    masks = nc.dram_tensor("masks", [128, 2, 128], F32, kind="ExternalInput")
    out = nc.dram_tensor("out", [NLC * 128, D], F32, kind="ExternalOutput")

    with tile.TileContext(nc) as tc:
        _body(nc, tc, x, wq, wk, wv, masks, out)
    nc.compile()
    return nc


def _body(nc, tc, x, wq, wk, wv, masks, out):
    with (
        tc.tile_pool(name="const", bufs=1) as const_pool,
        tc.tile_pool(name="dram", bufs=1, space="DRAM") as dram_pool,
        tc.tile_pool(name="xnat", bufs=NLC) as xnat_pool,
        tc.tile_pool(name="xt", bufs=NE) as xt_pool,
        tc.tile_pool(name="proj", bufs=1) as proj_pool,
        tc.tile_pool(name="escore", bufs=1) as e_pool,
        tc.tile_pool(name="zout", bufs=2) as zout_pool,
    ):
        # ---- constants ----
        ident = const_pool.tile([128, 128], BF16, name="ident")
        make_identity(nc, ident)
        wq_sb = const_pool.tile([128, NE, D], BF16, name="wq_sb")
        wk_sb = const_pool.tile([128, NE, D], BF16, name="wk_sb")
        wv_sb = const_pool.tile([128, NE, D], BF16, name="wv_sb")
        nc.gpsimd.dma_start(out=wq_sb[:], in_=wq[:].rearrange("(c p) d -> p c d", p=128))
        nc.gpsimd.dma_start(out=wk_sb[:], in_=wk[:].rearrange("(c p) d -> p c d", p=128))
        nc.gpsimd.dma_start(out=wv_sb[:], in_=wv[:].rearrange("(c p) d -> p c d", p=128))
        masks_sb = const_pool.tile([128, 2, 128], BF16, name="masks_sb")
        nc.gpsimd.dma_start(out=masks_sb[:], in_=masks[:])

        # ---- x load (cast f32 -> bf16 in DMA) ----
        x_sb = []
        for lc in range(NLC):
            xt_nat = xnat_pool.tile([128, E], BF16, name=f"xnat{lc}", tag="xnat")
            nc.gpsimd.dma_start(out=xt_nat[:], in_=x[ts(lc, 128), :])
            x_sb.append(xt_nat)

        # ---- transpose x: xT[e] = [128 E, 1024 t] ----
        xT = [xt_pool.tile([128, NLC * 128], BF16, name=f"xT{e}", tag="xt")
              for e in range(NE)]
        with tc.tile_pool(name="tp_psum", bufs=2, space="PSUM") as tp_psum:
            for e in range(NE):
                for half in range(2):
                    tp = tp_psum.tile([128, 512], BF16, tag="tp")
                    for j in range(4):
                        lc = half * 4 + j
                        nc.tensor.transpose(
                            out=tp[:, ts(j, 128)],
                            in_=x_sb[lc][:, ts(e, 128)],
                            identity=ident[:],
                        )
                    nc.vector.tensor_copy(out=xT[e][:, ts(half, 512)], in_=tp[:])

            # ---- projections ----
            qT_sb = proj_pool.tile([128, NLC * 128], BF16, name="qT_sb")
            kT_loc = proj_pool.tile([128, NLC * 128], BF16, name="kT_loc")
            vT_sb = proj_pool.tile([128, NLC * 128], BF16, name="vT_sb")
            v_loc = proj_pool.tile([128, NLC * 128], BF16, name="v_loc")
            with tc.tile_pool(name="pj_psum", bufs=1, space="PSUM") as pj_psum:
                for piece in range(2):
                    k_ps = pj_psum.tile([128, 512], F32, tag="k_ps")
                    for e in range(NE):
                        nc.tensor.matmul(
                            k_ps[:], lhsT=wk_sb[:, e, :],
                            rhs=xT[e][:, ts(piece, 512)],
                            start=(e == 0), stop=(e == NE - 1),
                        )
                    nc.scalar.copy(out=kT_loc[:, ts(piece, 512)], in_=k_ps[:])
                for piece in range(2):
                    q_ps = pj_psum.tile([128, 512], F32, tag="q_ps")
                    vt_ps = pj_psum.tile([128, 512], F32, tag="vt_ps")
                    for e in range(NE):
                        nc.tensor.matmul(
                            q_ps[:], lhsT=wq_sb[:, e, :],
                            rhs=xT[e][:, ts(piece, 512)],
                            start=(e == 0), stop=(e == NE - 1),
                        )
                        nc.tensor.matmul(
                            vt_ps[:], lhsT=wv_sb[:, e, :],
                            rhs=xT[e][:, ts(piece, 512)],
                            start=(e == 0), stop=(e == NE - 1),
                        )
                    nc.vector.tensor_copy(out=qT_sb[:, ts(piece, 512)], in_=q_ps[:])
                    nc.vector.tensor_copy(out=vT_sb[:, ts(piece, 512)], in_=vt_ps[:])

                # v natural [t, D] chunks via PE transpose of vT
                for half in range(2):
                    tpv = tp_psum.tile([128, 512], BF16, tag="tp")
                    for j in range(4):
                        lc = half * 4 + j
                        nc.tensor.transpose(
                            out=tpv[:, ts(j, 128)],
                            in_=vT_sb[:, ts(lc, 128)],
                            identity=ident[:],
                        )
                    nc.vector.tensor_copy(out=v_loc[:, ts(half, 512)], in_=tpv[:])

        # ---- AllGather kT & v across the pair ----
        cc_in = dram_pool.tile([128, 2048], BF16, name="cc_in")
        cc_out = dram_pool.tile([2, 128, 2048], BF16, name="cc_out")
        nc.sync.dma_start(out=cc_in[:, ds(0, 1024)], in_=kT_loc[:])
        nc.sync.dma_start(out=cc_in[:, ds(1024, 1024)], in_=v_loc[:])
        nc.gpsimd.collective_compute(
            "AllGather", ALU.bypass, replica_groups=REPLICA_GROUPS,
            ins=[cc_in[:].opt()], outs=[cc_out[:].opt()],
        )
        kT_full = proj_pool.tile([128, 2048], BF16, name="kT_full")
        v_full = proj_pool.tile([128, 2048], BF16, name="v_full")
        for r in range(2):
            nc.sync.dma_start(out=kT_full[:, ds(r * 1024, 1024)],
                              in_=cc_out[r, :, ds(0, 1024)])
            nc.sync.dma_start(out=v_full[:, ds(r * 1024, 1024)],
                              in_=cc_out[r, :, ds(1024, 1024)])

        # ---- scores / exp / normalizer partials ----
        stats = const_pool.tile([128, NSB * 4], F32, name="stats")
        nc.vector.memset(stats[:], 0.0)
        e_tiles = {}  # (sb, lc) -> AP [128 s, 128 t]
        with tc.tile_pool(name="sc_psum", bufs=2, space="PSUM") as sc_psum:
            for sb in range(NSB):
                lo = sb // 2
                kb = kT_full[:, ds(gpos(sb) * 128, 128)]
                # masked head piece (chunk that may straddle the diagonal)
                sc = sc_psum.tile([128, 128], F32, tag="scm")
                nc.tensor.matmul(sc[:], lhsT=kb, rhs=qT_sb[:, ds(lo * 128, 128)],
                                 start=True, stop=True)
                em = e_pool.tile([128, 128], BF16, name=f"em{sb}", tag=f"em{sb}")
                nc.scalar.activation(out=em[:], in_=sc[:], func=AF.Exp, scale=SCALE)
                nc.vector.tensor_tensor(out=em[:], in0=em[:],
                                        in1=masks_sb[:, sb % 2, :], op=ALU.mult)
                nc.vector.reduce_sum(out=stats[:, ds(sb * 4 + 3, 1)], in_=em[:],
                                     axis=AX.X)
                e_tiles[(sb, lo)] = em[:]
                # full pieces
                start_lc = lo + 1
                pidx = 0
                while start_lc < NLC:
                    n = min(4, NLC - start_lc)
                    scf = sc_psum.tile([128, 512], F32, tag="scf")
                    nc.tensor.matmul(
                        scf[:, ds(0, n * 128)], lhsT=kb,
                        rhs=qT_sb[:, ds(start_lc * 128, n * 128)],
                        start=True, stop=True,
                    )
                    ef = e_pool.tile([128, n * 128], BF16,
                                     name=f"ef{sb}_{pidx}", tag=f"ef{sb}_{pidx}")
                    nc.scalar.activation(
                        out=ef[:], in_=scf[:, ds(0, n * 128)], func=AF.Exp,
                        scale=SCALE, accum_out=stats[:, ds(sb * 4 + pidx, 1)],
                    )
                    for j in range(n):
                        e_tiles[(sb, start_lc + j)] = ef[:, ts(j, 128)]
                    start_lc += n
                    pidx += 1

            # ---- normalizer AllReduce + reciprocal + v scaling ----
            zsum_loc = const_pool.tile([128, NSB], F32, name="zsum_loc")
            for sb in range(NSB):
                nc.vector.reduce_sum(out=zsum_loc[:, ds(sb, 1)],
                                     in_=stats[:, ds(sb * 4, 4)], axis=AX.X)
            zin = dram_pool.tile([128, NSB], F32, name="zin")
            zout = dram_pool.tile([128, NSB], F32, name="zout_d")
            nc.sync.dma_start(out=zin[:], in_=zsum_loc[:])
            nc.gpsimd.collective_compute(
                "AllReduce", ALU.add, replica_groups=REPLICA_GROUPS,
                ins=[zin[:].opt()], outs=[zout[:].opt()],
            )
            zsum_full = const_pool.tile([128, NSB], F32, name="zsum_full")
            nc.sync.dma_start(out=zsum_full[:], in_=zout[:])
            recip = const_pool.tile([128, NSB], F32, name="recip")
            nc.vector.reciprocal(out=recip[:], in_=zsum_full[:])
            v_scaled = proj_pool.tile([128, 2048], BF16, name="v_scaled")
            for sb in range(NSB):
                gp = gpos(sb)
                nc.vector.tensor_scalar_mul(
                    out=v_scaled[:, ds(gp * 128, 128)],
                    in0=v_full[:, ds(gp * 128, 128)],
                    scalar1=recip[:, ds(sb, 1)],
                )

            # ---- z = A @ v' per local chunk ----
            with tc.tile_pool(name="av_psum", bufs=1, space="PSUM") as av_psum:
                for lc in range(NLC):
                    zp = av_psum.tile([128, D], F32, tag="zp")
                    nsb = 2 * lc + 2
                    for sb in range(nsb):
                        nc.tensor.matmul(
                            zp[:], lhsT=e_tiles[(sb, lc)],
                            rhs=v_scaled[:, ds(gpos(sb) * 128, 128)],
                            start=(sb == 0), stop=(sb == nsb - 1),
                        )
                    z_sb = zout_pool.tile([128, D], F32, tag="z_sb")
                    if lc % 2 == 0:
                        nc.vector.tensor_copy(out=z_sb[:], in_=zp[:])
                    else:
                        nc.scalar.copy(out=z_sb[:], in_=zp[:])
                    nc.sync.dma_start(out=out[ts(lc, 128), :], in_=z_sb[:])


_NC_CACHE = None


def _get_nc():
    global _NC_CACHE
    if _NC_CACHE is None:
        _NC_CACHE = build_nc()
    return _NC_CACHE


def _host_masks(h: int) -> np.ndarray:
    tri = (np.arange(128)[None, :] >= np.arange(128)[:, None]).astype(np.float32)
    ones = np.ones((128, 128), np.float32)
    zeros = np.zeros((128, 128), np.float32)
    pair = [tri, zeros] if h == 0 else [ones, tri]
    return np.ascontiguousarray(np.stack(pair, axis=0).transpose(1, 0, 2))


def kernel(x_in, Wq, Wk, Wv):
    x_in = np.asarray(x_in, dtype=np.float32)
    Wq = np.ascontiguousarray(np.asarray(Wq, dtype=np.float32))
    Wk = np.ascontiguousarray(np.asarray(Wk, dtype=np.float32))
    Wv = np.ascontiguousarray(np.asarray(Wv, dtype=np.float32))
    nc = _get_nc()
    in_maps = []
    for c in range(N_CORES):
        b, h = c // 2, c % 2
        rows = np.concatenate(
            [x_in[b, (2 * lc + h) * 128:(2 * lc + h + 1) * 128] for lc in range(NLC)]
        )
        in_maps.append({
            "x": np.ascontiguousarray(rows),
            "wq": Wq, "wk": Wk, "wv": Wv,
            "masks": _host_masks(h),
        })
    res = run_bass_kernel_spmd(nc, in_maps, core_ids=list(range(N_CORES)))
    out = np.empty((B, T, D), np.float32)
    for c in range(N_CORES):
        b, h = c // 2, c % 2
        o = res.results[c]["out"]
        for lc in range(NLC):
            g = 2 * lc + h
            out[b, g * 128:(g + 1) * 128] = o[lc * 128:(lc + 1) * 128]
    return out



# revision 10
# speedup vs baseline: 1.3323x; 1.3323x over previous
"""Trainium2 Bass kernel for nn_AttentionHead (softmax over query axis).

Sharding (8 cores = 4 batches x 2): core pair (2b, 2b+1) handles batch b.
Rank h = c%2 owns KEY blocks of parity h: local chunk lsb <-> global key
block gk = 2*lsb + h.  Query rows are staged identically (parity h rows),
so each core projects q/k/v for its own 1024 rows.

Per core (single SPMD program; h only appears in host-staged data):
  - host stages xT = x[rows].T in bf16 -> projections need no PE transposes
  - qT/kT/vT projections with W tiles as stationary operand
  - AllGather qT across the pair; remote queries fetched with a
    runtime-offset DMA (slot 1-h)
  - scores sT[s, t] = kb.T @ qT for all queries t >= key block, exp via
    ACT (scale 1/sqrt(128)); causal handled by ADDITIVE masks on the psum
    before exp (diag: tri of -1e30; first remote chunk: all -1e30 iff h=1)
  - Z[s] = sum_t E[s, t] is fully local (key-sharded!) -> no AllReduce
  - v' = v/Z, AV accumulates zT[d, t] partial over local key blocks
  - zT staged to DRAM in a global (gpos) column order via runtime-offset
    DMAs, ReduceScatter(add) over the pair splits the D dim: core h ends
    up with out[64, 2048] = z[d in 64h..64h+64, all t].
Host assembles the 8 [64, 2048] outputs into [4, 2048, 128].
"""
import sys

for _p in ("/opt/trn_rl_repo",):
    if _p not in sys.path:
        sys.path.append(_p)

import numpy as np
import ml_dtypes

import concourse.bass as bass
import concourse.mybir as mybir
import concourse.tile as tile
from concourse import bacc
from concourse.bass import ds, ts
from concourse.bass_utils import run_bass_kernel_spmd
from concourse.masks import make_identity

BF16 = mybir.dt.bfloat16
F32 = mybir.dt.float32
U32 = mybir.dt.uint32
AF = mybir.ActivationFunctionType
ALU = mybir.AluOpType
AX = mybir.AxisListType

B, T, E, D = 4, 2048, 2048, 128
NLC = 8          # local 128-chunks per core (queries and keys)
NE = 16          # E chunks of 128
SCALE = 1.0 / np.sqrt(D)
N_CORES = 8
REPLICA_GROUPS = [[0, 1], [2, 3], [4, 5], [6, 7]]
NEG = -1.0e30


def pieces(sb):
    """Column pieces [c0, width) of the valid query range [sb*128, 1024),
    split at absolute column 512 (PSUM-bank aligned)."""
    lo = sb * 128
    if lo < 512:
        return [(lo, 512 - lo), (512, 512)]
    return [(lo, 1024 - lo)]


def build_nc():
    nc = bacc.Bacc("TRN2", target_bir_lowering=False, debug=False,
                   num_devices=N_CORES)
    xT = nc.dram_tensor("xT", [E, NLC * 128], BF16, kind="ExternalInput")
    wq = nc.dram_tensor("wq", [128, NE, D], BF16, kind="ExternalInput")
    wk = nc.dram_tensor("wk", [128, NE, D], BF16, kind="ExternalInput")
    wv = nc.dram_tensor("wv", [128, NE, D], BF16, kind="ExternalInput")
    dmask = nc.dram_tensor("dmask", [128, 128], F32, kind="ExternalInput")
    rmask = nc.dram_tensor("rmask", [128, 128], F32, kind="ExternalInput")
    hoff = nc.dram_tensor("hoff", [1, 3], U32, kind="ExternalInput")
    out = nc.dram_tensor("out", [64, T], F32, kind="ExternalOutput")

    with tile.TileContext(nc) as tc:
        _body(nc, tc, xT, wq, wk, wv, dmask, rmask, hoff, out)
    nc.compile()
    return nc


def _body(nc, tc, xT, wq, wk, wv, dmask, rmask, hoff, out):
    with (
        tc.tile_pool(name="const", bufs=1) as const_pool,
        tc.tile_pool(name="dram", bufs=1, space="DRAM") as dram_pool,
        tc.tile_pool(name="proj", bufs=1) as proj_pool,
        tc.tile_pool(name="escore", bufs=1) as e_pool,
    ):
        # ---- constants / weights ----
        ident = const_pool.tile([128, 128], BF16, name="ident")
        make_identity(nc, ident)
        wq_sb = const_pool.tile([128, NE, D], BF16, name="wq_sb")
        wk_sb = const_pool.tile([128, NE, D], BF16, name="wk_sb")
        wv_sb = const_pool.tile([128, NE, D], BF16, name="wv_sb")
        nc.scalar.dma_start(out=wq_sb[:], in_=wq[:])
        nc.sync.dma_start(out=wk_sb[:], in_=wk[:])
        nc.sync.dma_start(out=wv_sb[:], in_=wv[:])
        dmask_sb = const_pool.tile([128, 128], F32, name="dmask_sb")
        rmask_sb = const_pool.tile([128, 128], F32, name="rmask_sb")
        nc.gpsimd.dma_start(out=dmask_sb[:], in_=dmask[:])
        nc.gpsimd.dma_start(out=rmask_sb[:], in_=rmask[:])
        stats = const_pool.tile([128, NLC * 4], F32, name="stats")
        nc.vector.memset(stats[:], 0.0)

        # runtime pair-rank offsets: hoff = [h*1024, (1-h)*1024, 1-h]
        _rh = nc.sync.alloc_register("rv_h")
        _rr = nc.sync.alloc_register("rv_r")
        _rs = nc.sync.alloc_register("rv_slot")
        nc.sync.reg_load(_rh, hoff[0:1, 0:1])
        nc.sync.reg_load(_rr, hoff[0:1, 1:2])
        nc.sync.reg_load(_rs, hoff[0:1, 2:3])
        rv_h = nc.sync.snap(_rh, donate=True, min_val=0, max_val=1024)
        rv_r = nc.sync.snap(_rr, donate=True, min_val=0, max_val=1024)
        rv_slot = nc.sync.snap(_rs, donate=True, min_val=0, max_val=1)

        # ---- x load (pre-transposed bf16, fully contiguous) ----
        x_sb = []
        for e in range(NE):
            t_x = const_pool.tile([128, NLC * 128], BF16, name=f"xT{e}")
            eng = nc.sync if e % 2 == 0 else nc.scalar
            eng.dma_start(out=t_x[:], in_=xT[ts(e, 128), :])
            x_sb.append(t_x)

        # ---- PE warmup (junk matmuls, keep HAM busy during DMA) ----
        with tc.tile_pool(name="wu_psum", bufs=1, space="PSUM") as wu_psum:
            junk = wu_psum.tile([128, 512], F32, name="junk")
            for i in range(8):
                nc.tensor.matmul(junk[:], lhsT=wq_sb[:, i, :],
                                 rhs=wq_sb[:, ds(0, 4), :],
                                 start=True, stop=True)

        qT_sb = proj_pool.tile([128, NLC * 128], BF16, name="qT_sb")
        kT_sb = proj_pool.tile([128, NLC * 128], BF16, name="kT_sb")
        vT_sb = proj_pool.tile([128, NLC * 128], BF16, name="vT_sb")
        q_rem = proj_pool.tile([128, NLC * 128], BF16, name="q_rem")
        v_loc = proj_pool.tile([128, NLC * 128], BF16, name="v_loc")
        v_sc = proj_pool.tile([128, NLC * 128], BF16, name="v_sc")
        recip = const_pool.tile([128, NLC], F32, name="recip")

        cc_qin = dram_pool.tile([128, 1024], BF16, name="cc_qin")
        cc_qout = dram_pool.tile([2, 128, 1024], BF16, name="cc_qout")

        with tc.tile_pool(name="pj_psum", bufs=1, space="PSUM") as pj:
            # ---- q & k projections, interleaved per e-chunk ----
            q0 = pj.tile([128, 512], F32, tag="p0")
            q1 = pj.tile([128, 512], F32, tag="p1")
            k0 = pj.tile([128, 512], F32, tag="k0")
            k1 = pj.tile([128, 512], F32, tag="k1")
            for e in range(NE):
                st, sp = (e == 0), (e == NE - 1)
                nc.tensor.matmul(q0[:], lhsT=wq_sb[:, e, :],
                                 rhs=x_sb[e][:, ds(0, 512)], start=st, stop=sp)
                nc.tensor.matmul(q1[:], lhsT=wq_sb[:, e, :],
                                 rhs=x_sb[e][:, ds(512, 512)], start=st, stop=sp)
                nc.tensor.matmul(k0[:], lhsT=wk_sb[:, e, :],
                                 rhs=x_sb[e][:, ds(0, 512)], start=st, stop=sp)
                nc.tensor.matmul(k1[:], lhsT=wk_sb[:, e, :],
                                 rhs=x_sb[e][:, ds(512, 512)], start=st, stop=sp)
            nc.vector.tensor_copy(out=qT_sb[:, ds(0, 512)], in_=q0[:])
            nc.vector.tensor_copy(out=qT_sb[:, ds(512, 512)], in_=q1[:])
            nc.vector.tensor_copy(out=kT_sb[:, ds(0, 512)], in_=k0[:])
            nc.vector.tensor_copy(out=kT_sb[:, ds(512, 512)], in_=k1[:])

            # ---- AllGather qT across the pair ----
            nc.sync.dma_start(out=cc_qin[:], in_=qT_sb[:])
            nc.gpsimd.collective_compute(
                "AllGather", ALU.bypass, replica_groups=REPLICA_GROUPS,
                ins=[cc_qin[:].opt()], outs=[cc_qout[:].opt()],
            )
            # fetch the PEER's slot (index 1-h) at runtime
            nc.sync.dma_start(out=q_rem[:], in_=cc_qout[ds(rv_slot, 1), :, :])

            # ---- v projection (reuses q psum tags) ----
            v0 = pj.tile([128, 512], F32, tag="p0")
            v1 = pj.tile([128, 512], F32, tag="p1")
            for e in range(NE):
                st, sp = (e == 0), (e == NE - 1)
                nc.tensor.matmul(v0[:], lhsT=wv_sb[:, e, :],
                                 rhs=x_sb[e][:, ds(0, 512)], start=st, stop=sp)
                nc.tensor.matmul(v1[:], lhsT=wv_sb[:, e, :],
                                 rhs=x_sb[e][:, ds(512, 512)], start=st, stop=sp)
            nc.vector.tensor_copy(out=vT_sb[:, ds(0, 512)], in_=v0[:])
            nc.vector.tensor_copy(out=vT_sb[:, ds(512, 512)], in_=v1[:])

            # ---- v natural [s, D] chunks via PE transpose ----
            with tc.tile_pool(name="tp_psum", bufs=2, space="PSUM") as tp:
                for lsb in range(NLC):
                    tpv = tp.tile([128, 128], BF16, tag="tp")
                    nc.tensor.transpose(out=tpv[:], in_=vT_sb[:, ts(lsb, 128)],
                                        identity=ident[:])
                    nc.vector.tensor_copy(out=v_loc[:, ts(lsb, 128)], in_=tpv[:])

        # ---- scores: local queries first (no AG dependency) ----
        e_tiles = []
        with (
            tc.tile_pool(name="sc_psum", bufs=3, space="PSUM") as scp,
            tc.tile_pool(name="zt_psum", bufs=1, space="PSUM") as ztp,
        ):
            for sb in range(NLC):
                esb = e_pool.tile([128, 2048], BF16, name=f"esb{sb}")
                e_tiles.append(esb)
                kb = kT_sb[:, ts(sb, 128)]
                for pi, (c0, pw) in enumerate(pieces(sb)):
                    sc = scp.tile([128, 512], F32, tag="sc")
                    nc.tensor.matmul(sc[:, ds(0, pw)], lhsT=kb,
                                     rhs=qT_sb[:, ds(c0, pw)],
                                     start=True, stop=True)
                    if pi == 0:  # diagonal chunk: additive tri mask
                        nc.vector.tensor_tensor(
                            out=sc[:, ds(0, 128)], in0=sc[:, ds(0, 128)],
                            in1=dmask_sb[:], op=ALU.add)
                    nc.scalar.activation(out=esb[:, ds(c0, pw)],
                                         in_=sc[:, ds(0, pw)],
                                         func=AF.Exp, scale=SCALE)
                    nc.vector.reduce_sum(out=stats[:, ds(sb * 4 + pi, 1)],
                                         in_=esb[:, ds(c0, pw)], axis=AX.X)

            # ---- remote queries + Z + v-scale + AV, streamed per sb ----
            zt = [ztp.tile([128, 512], F32, tag=f"zt{j}", name=f"zt{j}")
                  for j in range(4)]
            for sb in range(NLC):
                esb = e_tiles[sb]
                kb = kT_sb[:, ts(sb, 128)]
                for pi, (c0, pw) in enumerate(pieces(sb)):
                    sc = scp.tile([128, 512], F32, tag="sc")
                    nc.tensor.matmul(sc[:, ds(0, pw)], lhsT=kb,
                                     rhs=q_rem[:, ds(c0, pw)],
                                     start=True, stop=True)
                    if pi == 0:  # first remote chunk invalid iff h=1
                        nc.vector.tensor_tensor(
                            out=sc[:, ds(0, 128)], in0=sc[:, ds(0, 128)],
                            in1=rmask_sb[:], op=ALU.add)
                    nc.scalar.activation(out=esb[:, ds(1024 + c0, pw)],
                                         in_=sc[:, ds(0, pw)],
                                         func=AF.Exp, scale=SCALE)
                    nc.vector.reduce_sum(out=stats[:, ds(sb * 4 + 2 + pi, 1)],
                                         in_=esb[:, ds(1024 + c0, pw)], axis=AX.X)
                # Z (local!) -> recip -> v'
                zs = const_pool.tile([128, 1], F32, tag="zs")
                nc.vector.reduce_sum(out=zs[:], in_=stats[:, ds(sb * 4, 4)],
                                     axis=AX.X)
                nc.vector.reciprocal(out=recip[:, ds(sb, 1)], in_=zs[:])
                nc.vector.tensor_scalar_mul(
                    out=v_sc[:, ts(sb, 128)], in0=v_loc[:, ts(sb, 128)],
                    scalar1=recip[:, ds(sb, 1)])
                # AV: zT[d, t] += v'.T @ E  (cols: local 0..1023, remote 1024..)
                vs = v_sc[:, ts(sb, 128)]
                for half in range(2):
                    for (c0, pw) in pieces(sb):
                        j = half * 2 + (c0 // 512)
                        nc.tensor.matmul(
                            zt[j][:, ds(c0 % 512, pw)], lhsT=vs,
                            rhs=esb[:, ds(half * 1024 + c0, pw)],
                            start=(sb == 0), stop=(sb == NLC - 1),
                            skip_group_check=True)

            # ---- stage zT (swap halves into global gpos order) + RS ----
            zT_f = proj_pool.tile([128, 2048], F32, name="zT_f")
            for j in range(4):
                nc.vector.tensor_copy(out=zT_f[:, ds(j * 512, 512)], in_=zt[j][:])
            zin = dram_pool.tile([128, 2048], F32, name="zin")
            nc.sync.dma_start(out=zin[:, ds(rv_h, 1024)],
                              in_=zT_f[:, ds(0, 1024)])
            nc.sync.dma_start(out=zin[:, ds(rv_r, 1024)],
                              in_=zT_f[:, ds(1024, 1024)])
            zred = dram_pool.tile([64, 2048], F32, name="zred")
            nc.gpsimd.collective_compute(
                "ReduceScatter", ALU.add, replica_groups=REPLICA_GROUPS,
                ins=[zin[:].opt()], outs=[zred[:].opt()],
            )
            zred_sb = proj_pool.tile([64, 2048], F32, name="zred_sb")
            nc.scalar.dma_start(out=zred_sb[:], in_=zred[:])
            nc.scalar.dma_start(out=out[:], in_=zred_sb[:])


_NC_CACHE = None


def _get_nc():
    global _NC_CACHE
    if _NC_CACHE is None:
        _NC_CACHE = build_nc()
    return _NC_CACHE


def _w_tiles(W):
    return np.ascontiguousarray(
        np.asarray(W, np.float32).reshape(NE, 128, D).transpose(1, 0, 2)
    ).astype(ml_dtypes.bfloat16)


def make_in_maps(x_in, Wq, Wk, Wv):
    x_in = np.asarray(x_in, np.float32)
    wqt, wkt, wvt = _w_tiles(Wq), _w_tiles(Wk), _w_tiles(Wv)
    tri = np.where(np.arange(128)[None, :] >= np.arange(128)[:, None],
                   0.0, NEG).astype(np.float32)
    in_maps = []
    for c in range(N_CORES):
        b, h = c // 2, c % 2
        rows = np.concatenate(
            [np.arange((2 * lc + h) * 128, (2 * lc + h + 1) * 128)
             for lc in range(NLC)])
        xTc = np.ascontiguousarray(x_in[b][rows].T).astype(ml_dtypes.bfloat16)
        rmask = (np.zeros((128, 128), np.float32) if h == 0
                 else np.full((128, 128), NEG, np.float32))
        in_maps.append({
            "xT": xTc, "wq": wqt, "wk": wkt, "wv": wvt,
            "dmask": tri, "rmask": rmask,
            "hoff": np.array([[h * 1024, (1 - h) * 1024, 1 - h]], np.uint32),
        })
    return in_maps


def assemble(results):
    z = np.empty((B, T, D), np.float32)
    for c in range(N_CORES):
        b, h = c // 2, c % 2
        o = results[c]["out"]  # [64, 2048] gpos cols
        for p in range(16):
            g = 2 * (p % 8) + (p // 8)
            z[b, g * 128:(g + 1) * 128, 64 * h:64 * h + 64] = \
                o[:, p * 128:(p + 1) * 128].T
    return z


def kernel(x_in, Wq, Wk, Wv):
    nc = _get_nc()
    in_maps = make_in_maps(x_in, Wq, Wk, Wv)
    res = run_bass_kernel_spmd(nc, in_maps, core_ids=list(range(N_CORES)))
    return assemble(res.results)


# revision 14
# speedup vs baseline: 1.3402x; 1.0059x over previous
"""Trainium2 Bass kernel for nn_AttentionHead (softmax over query axis).

Sharding (8 cores = 4 batches x 2): core pair (2b, 2b+1) handles batch b.
Rank h = c%2 owns KEY blocks of parity h: local chunk sb <-> global key
block gk = 2*sb + h.  Each core projects q for ALL 2048 rows (redundant,
avoids a mid-kernel AllGather) and k/v for its own 1024 rows only.

Per core (single SPMD program; h only appears in host-staged data):
  - host stages xT = x.T in bf16, columns [my 1024 rows | other 1024 rows]
    -> projections need no PE transposes and no gather
  - scores sT[s, t] = kb.T @ qT for all queries t >= key block, exp via
    ACT (scale 1/sqrt(128)); causal handled by ADDITIVE masks on the psum
    before exp (diag: tri of -1e30; first remote chunk: all -1e30 iff h=1)
  - Z[s] = sum_t E[s, t] is fully local (key-sharded) -> no AllReduce
  - v' = v/Z; AV accumulates zT[d, t] partials over local key blocks
  - zT (bf16) staged to DRAM in a global (gpos) column order via
    runtime-offset DMAs, ReduceScatter(add) over the pair splits the D
    dim: core h ends with out[64, 2048] = z[d in 64h..64h+64, all t].
Host assembles the 8 [64, 2048] outputs into [4, 2048, 128].
"""
import sys

for _p in ("/opt/trn_rl_repo",):
    if _p not in sys.path:
        sys.path.append(_p)

import numpy as np
import ml_dtypes

import concourse.bass as bass
import concourse.mybir as mybir
import concourse.tile as tile
from concourse import bacc
from concourse.bass import ds, ts
from concourse.bass_utils import run_bass_kernel_spmd
from concourse.masks import make_identity

BF16 = mybir.dt.bfloat16
F32 = mybir.dt.float32
U32 = mybir.dt.uint32
AF = mybir.ActivationFunctionType
ALU = mybir.AluOpType
AX = mybir.AxisListType

B, T, E, D = 4, 2048, 2048, 128
NLC = 8          # local 128-chunks per core (keys); queries = 2*NLC chunks
NE = 16          # E chunks of 128
SCALE = 1.0 / np.sqrt(D)
N_CORES = 8
REPLICA_GROUPS = [[0, 1], [2, 3], [4, 5], [6, 7]]
NEG = -1.0e30


def pieces(sb):
    """Column pieces [c0, width) of the valid query range [sb*128, 1024),
    split at absolute column 512 (PSUM-bank aligned)."""
    lo = sb * 128
    if lo < 512:
        return [(lo, 512 - lo), (512, 512)]
    return [(lo, 1024 - lo)]


def build_nc():
    nc = bacc.Bacc("TRN2", target_bir_lowering=False, debug=False,
                   num_devices=N_CORES)
    xT = nc.dram_tensor("xT", [E, T], BF16, kind="ExternalInput")
    wq = nc.dram_tensor("wq", [128, NE, D], BF16, kind="ExternalInput")
    wk = nc.dram_tensor("wk", [128, NE, D], BF16, kind="ExternalInput")
    wv = nc.dram_tensor("wv", [128, NE, D], BF16, kind="ExternalInput")
    dmask = nc.dram_tensor("dmask", [128, 128], F32, kind="ExternalInput")
    rmask = nc.dram_tensor("rmask", [128, 128], F32, kind="ExternalInput")
    hoff = nc.dram_tensor("hoff", [1, 2], U32, kind="ExternalInput")
    out = nc.dram_tensor("out", [64, T], F32, kind="ExternalOutput")

    with tile.TileContext(nc) as tc:
        _body(nc, tc, xT, wq, wk, wv, dmask, rmask, hoff, out)
    nc.compile()
    return nc


def _body(nc, tc, xT, wq, wk, wv, dmask, rmask, hoff, out):
    with (
        tc.tile_pool(name="const", bufs=1) as const_pool,
        tc.tile_pool(name="dram", bufs=1, space="DRAM") as dram_pool,
        tc.tile_pool(name="proj", bufs=1) as proj_pool,
        tc.tile_pool(name="escore", bufs=1) as e_pool,
    ):
        # ---- constants / weights ----
        ident = const_pool.tile([128, 128], BF16, name="ident")
        make_identity(nc, ident)
        wq_sb = const_pool.tile([128, NE, D], BF16, name="wq_sb")
        wk_sb = const_pool.tile([128, NE, D], BF16, name="wk_sb")
        wv_sb = const_pool.tile([128, NE, D], BF16, name="wv_sb")
        nc.scalar.dma_start(out=wq_sb[:], in_=wq[:])
        nc.sync.dma_start(out=wk_sb[:], in_=wk[:])
        nc.sync.dma_start(out=wv_sb[:], in_=wv[:])
        dmask_sb = const_pool.tile([128, 128], F32, name="dmask_sb")
        rmask_sb = const_pool.tile([128, 128], F32, name="rmask_sb")
        nc.gpsimd.dma_start(out=dmask_sb[:], in_=dmask[:])
        nc.gpsimd.dma_start(out=rmask_sb[:], in_=rmask[:])
        stats = const_pool.tile([128, NLC * 4], F32, name="stats")
        nc.vector.memset(stats[:], 0.0)

        # runtime pair-rank offsets for zin staging: hoff = [h*1024, (1-h)*1024]
        _rh = nc.sync.alloc_register("rv_h")
        _rr = nc.sync.alloc_register("rv_r")
        nc.sync.reg_load(_rh, hoff[0:1, 0:1])
        nc.sync.reg_load(_rr, hoff[0:1, 1:2])
        rv_h = nc.sync.snap(_rh, donate=True, min_val=0, max_val=1024)
        rv_r = nc.sync.snap(_rr, donate=True, min_val=0, max_val=1024)

        # ---- x load (pre-transposed bf16, fully contiguous) ----
        x_sb = []
        for e in range(NE):
            t_x = const_pool.tile([128, T], BF16, name=f"xT{e}")
            eng = nc.sync if e % 2 == 0 else nc.scalar
            eng.dma_start(out=t_x[:], in_=xT[ts(e, 128), :])
            x_sb.append(t_x)

        qT_sb = proj_pool.tile([128, T], BF16, name="qT_sb")
        kT_sb = proj_pool.tile([128, NLC * 128], BF16, name="kT_sb")
        vT_sb = proj_pool.tile([128, NLC * 128], BF16, name="vT_sb")
        v_loc = proj_pool.tile([128, NLC * 128], BF16, name="v_loc")
        v_sc = proj_pool.tile([128, NLC * 128], BF16, name="v_sc")
        recip = const_pool.tile([128, NLC], F32, name="recip")

        with tc.tile_pool(name="pj_psum", bufs=1, space="PSUM") as pj:
            # ---- PE warmup junk (shares the q0 bank, runs first) ----
            junk = pj.tile([128, 512], F32, tag="p0")
            for i in range(6):
                nc.tensor.matmul(junk[:], lhsT=wq_sb[:, i, :],
                                 rhs=wq_sb[:, ds(0, 4), :],
                                 start=True, stop=True)

            # ---- q (all 2048 queries) / k / v projections, per e-chunk ----
            qp = [pj.tile([128, 512], F32, tag=f"p{i}", name=f"qp{i}")
                  for i in range(4)]
            k0 = pj.tile([128, 512], F32, tag="k0")
            k1 = pj.tile([128, 512], F32, tag="k1")
            v0 = pj.tile([128, 512], F32, tag="v0")
            v1 = pj.tile([128, 512], F32, tag="v1")
            for e in range(NE):
                st, sp = (e == 0), (e == NE - 1)
                for i in range(4):
                    nc.tensor.matmul(qp[i][:], lhsT=wq_sb[:, e, :],
                                     rhs=x_sb[e][:, ds(i * 512, 512)],
                                     start=st, stop=sp)
                nc.tensor.matmul(k0[:], lhsT=wk_sb[:, e, :],
                                 rhs=x_sb[e][:, ds(0, 512)], start=st, stop=sp)
                nc.tensor.matmul(k1[:], lhsT=wk_sb[:, e, :],
                                 rhs=x_sb[e][:, ds(512, 512)], start=st, stop=sp)
                nc.tensor.matmul(v0[:], lhsT=wv_sb[:, e, :],
                                 rhs=x_sb[e][:, ds(0, 512)], start=st, stop=sp)
                nc.tensor.matmul(v1[:], lhsT=wv_sb[:, e, :],
                                 rhs=x_sb[e][:, ds(512, 512)], start=st, stop=sp)
            nc.vector.tensor_copy(out=kT_sb[:, ds(0, 512)], in_=k0[:])
            nc.vector.tensor_copy(out=kT_sb[:, ds(512, 512)], in_=k1[:])
            for i in range(4):
                nc.vector.tensor_copy(out=qT_sb[:, ds(i * 512, 512)], in_=qp[i][:])
            nc.vector.tensor_copy(out=vT_sb[:, ds(0, 512)], in_=v0[:])
            nc.vector.tensor_copy(out=vT_sb[:, ds(512, 512)], in_=v1[:])

        # ---- v natural [s, D] chunks via PE transpose ----
        with tc.tile_pool(name="tp_psum", bufs=2, space="PSUM") as tp:
            for sb in range(NLC):
                tpv = tp.tile([128, 128], BF16, tag="tp")
                nc.tensor.transpose(out=tpv[:], in_=vT_sb[:, ts(sb, 128)],
                                    identity=ident[:])
                nc.vector.tensor_copy(out=v_loc[:, ts(sb, 128)], in_=tpv[:])

        # ---- scores / exp / Z / v-scale / AV, streamed per key block ----
        with (
            tc.tile_pool(name="sc_psum", bufs=2, space="PSUM") as scp,
            tc.tile_pool(name="zt_psum", bufs=1, space="PSUM") as ztp,
        ):
            zt = [ztp.tile([128, 512], F32, tag=f"zt{j}", name=f"zt{j}")
                  for j in range(4)]
            for sb in range(NLC):
                esb = e_pool.tile([128, 2048], BF16, name=f"esb{sb}")
                kb = kT_sb[:, ts(sb, 128)]
                for half in range(2):  # 0: my queries, 1: peer's queries
                    mask = dmask_sb if half == 0 else rmask_sb
                    for pi, (c0, pw) in enumerate(pieces(sb)):
                        sc = scp.tile([128, 512], F32, tag="sc")
                        nc.tensor.matmul(sc[:, ds(0, pw)], lhsT=kb,
                                         rhs=qT_sb[:, ds(half * 1024 + c0, pw)],
                                         start=True, stop=True)
                        if pi == 0:  # chunk on/left of the causal boundary
                            nc.vector.tensor_tensor(
                                out=sc[:, ds(0, 128)], in0=sc[:, ds(0, 128)],
                                in1=mask[:], op=ALU.add)
                        nc.scalar.activation(out=esb[:, ds(half * 1024 + c0, pw)],
                                             in_=sc[:, ds(0, pw)],
                                             func=AF.Exp, scale=SCALE)
                        nc.vector.reduce_sum(
                            out=stats[:, ds(sb * 4 + half * 2 + pi, 1)],
                            in_=esb[:, ds(half * 1024 + c0, pw)], axis=AX.X)
                # Z (local!) -> recip -> v'
                zs = const_pool.tile([128, 1], F32, tag="zs")
                nc.vector.reduce_sum(out=zs[:], in_=stats[:, ds(sb * 4, 4)],
                                     axis=AX.X)
                nc.vector.reciprocal(out=recip[:, ds(sb, 1)], in_=zs[:])
                nc.vector.tensor_scalar_mul(
                    out=v_sc[:, ts(sb, 128)], in0=v_loc[:, ts(sb, 128)],
                    scalar1=recip[:, ds(sb, 1)])
                # AV: zT[d, t] += v'.T @ E  (cols: local 0..1023, remote 1024..)
                vs = v_sc[:, ts(sb, 128)]
                for half in range(2):
                    for (c0, pw) in pieces(sb):
                        j = half * 2 + (c0 // 512)
                        nc.tensor.matmul(
                            zt[j][:, ds(c0 % 512, pw)], lhsT=vs,
                            rhs=esb[:, ds(half * 1024 + c0, pw)],
                            start=(sb == 0), stop=(sb == NLC - 1),
                            skip_group_check=True)

            # ---- stage zT (swap halves into global gpos order) + RS ----
            zT_f = proj_pool.tile([128, 2048], BF16, name="zT_f")
            for j in range(4):
                nc.vector.tensor_copy(out=zT_f[:, ds(j * 512, 512)], in_=zt[j][:])
            zin = dram_pool.tile([128, 2048], BF16, name="zin")
            nc.sync.dma_start(out=zin[:, ds(rv_h, 1024)],
                              in_=zT_f[:, ds(0, 1024)])
            nc.sync.dma_start(out=zin[:, ds(rv_r, 1024)],
                              in_=zT_f[:, ds(1024, 1024)])
            zred = dram_pool.tile([64, 2048], BF16, name="zred")
            nc.gpsimd.collective_compute(
                "ReduceScatter", ALU.add, replica_groups=REPLICA_GROUPS,
                ins=[zin[:].opt()], outs=[zred[:].opt()],
            )
            nc.gpsimd.dma_start(out=out[:], in_=zred[:])


_NC_CACHE = None


def _get_nc():
    global _NC_CACHE
    if _NC_CACHE is None:
        _NC_CACHE = build_nc()
    return _NC_CACHE


def _w_tiles(W):
    return np.ascontiguousarray(
        np.asarray(W, np.float32).reshape(NE, 128, D).transpose(1, 0, 2)
    ).astype(ml_dtypes.bfloat16)


def make_in_maps(x_in, Wq, Wk, Wv):
    x_in = np.asarray(x_in, np.float32)
    wqt, wkt, wvt = _w_tiles(Wq), _w_tiles(Wk), _w_tiles(Wv)
    tri = np.where(np.arange(128)[None, :] >= np.arange(128)[:, None],
                   0.0, NEG).astype(np.float32)
    in_maps = []
    for c in range(N_CORES):
        b, h = c // 2, c % 2
        mine = np.concatenate(
            [np.arange((2 * lc + h) * 128, (2 * lc + h + 1) * 128)
             for lc in range(NLC)])
        other = np.concatenate(
            [np.arange((2 * lc + 1 - h) * 128, (2 * lc + 2 - h) * 128)
             for lc in range(NLC)])
        xTc = np.ascontiguousarray(
            x_in[b][np.concatenate([mine, other])].T).astype(ml_dtypes.bfloat16)
        rmask = (np.zeros((128, 128), np.float32) if h == 0
                 else np.full((128, 128), NEG, np.float32))
        in_maps.append({
            "xT": xTc, "wq": wqt, "wk": wkt, "wv": wvt,
            "dmask": tri, "rmask": rmask,
            "hoff": np.array([[h * 1024, (1 - h) * 1024]], np.uint32),
        })
    return in_maps


def assemble(results):
    z = np.empty((B, T, D), np.float32)
    for c in range(N_CORES):
        b, h = c // 2, c % 2
        o = results[c]["out"]  # [64, 2048] gpos cols
        for p in range(16):
            g = 2 * (p % 8) + (p // 8)
            z[b, g * 128:(g + 1) * 128, 64 * h:64 * h + 64] = \
                o[:, p * 128:(p + 1) * 128].T
    return z


def kernel(x_in, Wq, Wk, Wv):
    nc = _get_nc()
    in_maps = make_in_maps(x_in, Wq, Wk, Wv)
    res = run_bass_kernel_spmd(nc, in_maps, core_ids=list(range(N_CORES)))
    return assemble(res.results)


# revision 20
# speedup vs baseline: 1.5499x; 1.1565x over previous
"""Trainium2 Bass kernel for nn_AttentionHead (softmax over query axis).

Sharding (8 cores = 4 batches x 2): core pair (2b, 2b+1) handles batch b.
Rank h = c%2 owns KEY blocks of parity h: local chunk sb <-> global key
block gk = 2*sb + h.  Each core projects q for ALL 2048 rows (redundant,
avoids a mid-kernel AllGather) and k/v for its own 1024 rows only.

Per core (single SPMD program; h only appears in host-staged data):
  - host stages xT = x.T in bf16, columns [my 1024 rows | other 1024 rows]
    -> projections need no PE transposes and no gather
  - scores sT[s, t] = kb.T @ qT for all queries t >= key block, exp via
    ACT (scale 1/sqrt(128)); causal handled by ADDITIVE masks on the psum
    before exp (diag: tri of -1e30; first remote chunk: all -1e30 iff h=1)
  - Z[s] = sum_t E[s, t] is fully local (key-sharded) -> no AllReduce
  - v' = v/Z; AV accumulates zT[d, t] partials over local key blocks
  - zT (bf16) staged to DRAM in a global (gpos) column order via
    runtime-offset DMAs, ReduceScatter(add) over the pair splits the D
    dim: core h ends with out[64, 2048] = z[d in 64h..64h+64, all t].
Host assembles the 8 [64, 2048] outputs into [4, 2048, 128].
"""
import sys

for _p in ("/opt/trn_rl_repo",):
    if _p not in sys.path:
        sys.path.append(_p)

import numpy as np
import ml_dtypes

import concourse.bass as bass
import concourse.mybir as mybir
import concourse.tile as tile
from concourse import bacc
from concourse.bass import ds, ts
from concourse.bass_utils import run_bass_kernel_spmd
from concourse.masks import make_identity

BF16 = mybir.dt.bfloat16
F32 = mybir.dt.float32
U32 = mybir.dt.uint32
AF = mybir.ActivationFunctionType
ALU = mybir.AluOpType
AX = mybir.AxisListType

B, T, E, D = 4, 2048, 2048, 128
NLC = 8          # local 128-chunks per core (keys); queries = 2*NLC chunks
NE = 16          # E chunks of 128
SCALE = 1.0 / np.sqrt(D)
N_CORES = 8
REPLICA_GROUPS = [[0, 1], [2, 3], [4, 5], [6, 7]]
NEG = -1.0e30


def pieces(sb):
    """Column pieces [c0, width) of the valid query range [sb*128, 1024),
    split at absolute column 512 (PSUM-bank aligned)."""
    lo = sb * 128
    if lo < 512:
        return [(lo, 512 - lo), (512, 512)]
    return [(lo, 1024 - lo)]


def build_nc():
    nc = bacc.Bacc("TRN2", target_bir_lowering=False, debug=False,
                   num_devices=N_CORES)
    xT = nc.dram_tensor("xT", [E, T], BF16, kind="ExternalInput")
    wq = nc.dram_tensor("wq", [128, NE, D], BF16, kind="ExternalInput")
    wk = nc.dram_tensor("wk", [128, NE, D], BF16, kind="ExternalInput")
    wv = nc.dram_tensor("wv", [128, NE, D], BF16, kind="ExternalInput")
    dmask = nc.dram_tensor("dmask", [128, 128], F32, kind="ExternalInput")
    rmask = nc.dram_tensor("rmask", [128, 128], F32, kind="ExternalInput")
    hoff = nc.dram_tensor("hoff", [1, 2], U32, kind="ExternalInput")
    out = nc.dram_tensor("out", [64, T], BF16, kind="ExternalOutput")

    with tile.TileContext(nc) as tc:
        _body(nc, tc, xT, wq, wk, wv, dmask, rmask, hoff, out)
    nc.compile()
    return nc


def _body(nc, tc, xT, wq, wk, wv, dmask, rmask, hoff, out):
    with (
        tc.tile_pool(name="const", bufs=1) as const_pool,
        tc.tile_pool(name="dram", bufs=1, space="DRAM") as dram_pool,
        tc.tile_pool(name="proj", bufs=1) as proj_pool,
        tc.tile_pool(name="escore", bufs=1) as e_pool,
    ):
        # ---- constants / weights ----
        ident = const_pool.tile([128, 128], BF16, name="ident")
        make_identity(nc, ident)
        wq_sb = const_pool.tile([128, NE, D], BF16, name="wq_sb")
        wk_sb = const_pool.tile([128, NE, D], BF16, name="wk_sb")
        wv_sb = const_pool.tile([128, NE, D], BF16, name="wv_sb")
        nc.scalar.dma_start(out=wq_sb[:], in_=wq[:])
        nc.sync.dma_start(out=wk_sb[:], in_=wk[:])
        nc.sync.dma_start(out=wv_sb[:], in_=wv[:])
        dmask_sb = const_pool.tile([128, 128], F32, name="dmask_sb")
        rmask_sb = const_pool.tile([128, 128], F32, name="rmask_sb")
        nc.gpsimd.dma_start(out=dmask_sb[:], in_=dmask[:])
        nc.gpsimd.dma_start(out=rmask_sb[:], in_=rmask[:])
        stats = const_pool.tile([128, NLC * 4], F32, name="stats")
        nc.vector.memset(stats[:], 0.0)

        # ---- dummy early collective: absorbs the cc-stream entry barrier
        # and the ~11.5us first-op trigger latency while compute runs ----
        dummy_sb = const_pool.tile([128, 16], F32, name="dummy_sb")
        nc.vector.memset(dummy_sb[:], 0.0)
        dummy_in = dram_pool.tile([128, 16], F32, name="dummy_in")
        dummy_out = dram_pool.tile([2, 128, 16], F32, name="dummy_out")
        nc.gpsimd.dma_start(out=dummy_in[:], in_=dummy_sb[:])
        nc.gpsimd.collective_compute(
            "AllGather", ALU.bypass, replica_groups=REPLICA_GROUPS,
            ins=[dummy_in[:].opt()], outs=[dummy_out[:].opt()],
        )

        # runtime pair-rank offsets for zin staging: hoff = [h*1024, (1-h)*1024]
        _rh = nc.sync.alloc_register("rv_h")
        _rr = nc.sync.alloc_register("rv_r")
        nc.sync.reg_load(_rh, hoff[0:1, 0:1])
        nc.sync.reg_load(_rr, hoff[0:1, 1:2])
        rv_h = nc.sync.snap(_rh, donate=True, min_val=0, max_val=1024)
        rv_r = nc.sync.snap(_rr, donate=True, min_val=0, max_val=1024)

        # ---- x load (pre-transposed bf16, fully contiguous) ----
        x_sb = []
        for e in range(NE):
            t_x = const_pool.tile([128, T], BF16, name=f"xT{e}")
            nc.sync.dma_start(out=t_x[:, ds(0, 1024)],
                              in_=xT[ts(e, 128), ds(0, 1024)])
            nc.scalar.dma_start(out=t_x[:, ds(1024, 1024)],
                                in_=xT[ts(e, 128), ds(1024, 1024)])
            x_sb.append(t_x)

        qT_sb = proj_pool.tile([128, T], BF16, name="qT_sb")
        kT_sb = proj_pool.tile([128, NLC * 128], BF16, name="kT_sb")
        vT_sb = proj_pool.tile([128, NLC * 128], BF16, name="vT_sb")
        v_loc = proj_pool.tile([128, NLC * 128], BF16, name="v_loc")
        v_sc = proj_pool.tile([128, NLC * 128], BF16, name="v_sc")
        recip = const_pool.tile([128, NLC], F32, name="recip")

        with tc.tile_pool(name="pj_psum", bufs=1, space="PSUM") as pj:
            # ---- PE warmup junk (shares the q0 bank, runs first) ----
            junk = pj.tile([128, 512], F32, tag="p0")
            for i in range(6):
                nc.tensor.matmul(junk[:], lhsT=wq_sb[:, i, :],
                                 rhs=wq_sb[:, ds(0, 4), :],
                                 start=True, stop=True)

            # ---- q (all 2048 queries) / k / v projections, per e-chunk ----
            qp = [pj.tile([128, 512], F32, tag=f"p{i}", name=f"qp{i}")
                  for i in range(4)]
            k0 = pj.tile([128, 512], F32, tag="k0")
            k1 = pj.tile([128, 512], F32, tag="k1")
            v0 = pj.tile([128, 512], F32, tag="v0")
            v1 = pj.tile([128, 512], F32, tag="v1")
            for e in range(NE):
                st, sp = (e == 0), (e == NE - 1)
                for i in range(4):
                    nc.tensor.matmul(qp[i][:], lhsT=wq_sb[:, e, :],
                                     rhs=x_sb[e][:, ds(i * 512, 512)],
                                     start=st, stop=sp)
                nc.tensor.matmul(k0[:], lhsT=wk_sb[:, e, :],
                                 rhs=x_sb[e][:, ds(0, 512)], start=st, stop=sp)
                nc.tensor.matmul(k1[:], lhsT=wk_sb[:, e, :],
                                 rhs=x_sb[e][:, ds(512, 512)], start=st, stop=sp)
                nc.tensor.matmul(v0[:], lhsT=wv_sb[:, e, :],
                                 rhs=x_sb[e][:, ds(0, 512)], start=st, stop=sp)
                nc.tensor.matmul(v1[:], lhsT=wv_sb[:, e, :],
                                 rhs=x_sb[e][:, ds(512, 512)], start=st, stop=sp)
            nc.vector.tensor_copy(out=kT_sb[:, ds(0, 512)], in_=k0[:])
            nc.vector.tensor_copy(out=kT_sb[:, ds(512, 512)], in_=k1[:])
            for i in range(4):
                nc.vector.tensor_copy(out=qT_sb[:, ds(i * 512, 512)], in_=qp[i][:])
            nc.vector.tensor_copy(out=vT_sb[:, ds(0, 512)], in_=v0[:])
            nc.vector.tensor_copy(out=vT_sb[:, ds(512, 512)], in_=v1[:])

        # ---- v natural [s, D] chunks via PE transpose ----
        with tc.tile_pool(name="tp_psum", bufs=2, space="PSUM") as tp:
            for sb in range(NLC):
                tpv = tp.tile([128, 128], BF16, tag="tp")
                nc.tensor.transpose(out=tpv[:], in_=vT_sb[:, ts(sb, 128)],
                                    identity=ident[:])
                nc.vector.tensor_copy(out=v_loc[:, ts(sb, 128)], in_=tpv[:])

        # ---- scores / exp / Z / v-scale / AV, streamed per key block ----
        with (
            tc.tile_pool(name="sc_psum", bufs=4, space="PSUM") as scp,
            tc.tile_pool(name="zt_psum", bufs=1, space="PSUM") as ztp,
        ):
            zt = [ztp.tile([128, 512], F32, tag=f"zt{j}", name=f"zt{j}")
                  for j in range(4)]
            for sb in range(NLC):
                esb = e_pool.tile([128, 2048], BF16, name=f"esb{sb}")
                kb = kT_sb[:, ts(sb, 128)]
                for half in range(2):  # 0: my queries, 1: peer's queries
                    mask = dmask_sb if half == 0 else rmask_sb
                    for pi, (c0, pw) in enumerate(pieces(sb)):
                        sc = scp.tile([128, 512], F32, tag="sc")
                        nc.tensor.matmul(sc[:, ds(0, pw)], lhsT=kb,
                                         rhs=qT_sb[:, ds(half * 1024 + c0, pw)],
                                         start=True, stop=True)
                        if pi == 0:  # chunk on/left of the causal boundary
                            nc.vector.tensor_tensor(
                                out=sc[:, ds(0, 128)], in0=sc[:, ds(0, 128)],
                                in1=mask[:], op=ALU.add)
                        nc.scalar.activation(out=esb[:, ds(half * 1024 + c0, pw)],
                                             in_=sc[:, ds(0, pw)],
                                             func=AF.Exp, scale=SCALE)
                        nc.vector.reduce_sum(
                            out=stats[:, ds(sb * 4 + half * 2 + pi, 1)],
                            in_=esb[:, ds(half * 1024 + c0, pw)], axis=AX.X)
                # Z (local!) -> recip -> v'
                zs = const_pool.tile([128, 1], F32, tag="zs")
                nc.vector.reduce_sum(out=zs[:], in_=stats[:, ds(sb * 4, 4)],
                                     axis=AX.X)
                nc.vector.reciprocal(out=recip[:, ds(sb, 1)], in_=zs[:])
                nc.vector.tensor_scalar_mul(
                    out=v_sc[:, ts(sb, 128)], in0=v_loc[:, ts(sb, 128)],
                    scalar1=recip[:, ds(sb, 1)])
                # AV: zT[d, t] += v'.T @ E  (cols: local 0..1023, remote 1024..)
                vs = v_sc[:, ts(sb, 128)]
                for half in range(2):
                    for (c0, pw) in pieces(sb):
                        j = half * 2 + (c0 // 512)
                        nc.tensor.matmul(
                            zt[j][:, ds(c0 % 512, pw)], lhsT=vs,
                            rhs=esb[:, ds(half * 1024 + c0, pw)],
                            start=(sb == 0), stop=(sb == NLC - 1),
                            skip_group_check=True)

            # ---- stage zT (swap halves into global gpos order) + RS ----
            zT_f = proj_pool.tile([128, 2048], BF16, name="zT_f")
            for j in range(4):
                nc.vector.tensor_copy(out=zT_f[:, ds(j * 512, 512)], in_=zt[j][:])
            zin = dram_pool.tile([128, 2048], BF16, name="zin")
            nc.sync.dma_start(out=zin[:, ds(rv_h, 1024)],
                              in_=zT_f[:, ds(0, 1024)])
            nc.sync.dma_start(out=zin[:, ds(rv_r, 1024)],
                              in_=zT_f[:, ds(1024, 1024)])
            zred = dram_pool.tile([64, 2048], BF16, name="zred")
            nc.gpsimd.collective_compute(
                "ReduceScatter", ALU.add, replica_groups=REPLICA_GROUPS,
                ins=[zin[:].opt()], outs=[zred[:].opt()],
            )
            nc.sync.dma_start(out=out[:], in_=zred[:])


_NC_CACHE = None


def _get_nc():
    global _NC_CACHE
    if _NC_CACHE is None:
        _NC_CACHE = build_nc()
    return _NC_CACHE


def _w_tiles(W):
    return np.ascontiguousarray(
        np.asarray(W, np.float32).reshape(NE, 128, D).transpose(1, 0, 2)
    ).astype(ml_dtypes.bfloat16)


def make_in_maps(x_in, Wq, Wk, Wv):
    x_in = np.asarray(x_in, np.float32)
    wqt, wkt, wvt = _w_tiles(Wq), _w_tiles(Wk), _w_tiles(Wv)
    tri = np.where(np.arange(128)[None, :] >= np.arange(128)[:, None],
                   0.0, NEG).astype(np.float32)
    in_maps = []
    for c in range(N_CORES):
        b, h = c // 2, c % 2
        mine = np.concatenate(
            [np.arange((2 * lc + h) * 128, (2 * lc + h + 1) * 128)
             for lc in range(NLC)])
        other = np.concatenate(
            [np.arange((2 * lc + 1 - h) * 128, (2 * lc + 2 - h) * 128)
             for lc in range(NLC)])
        xTc = np.ascontiguousarray(
            x_in[b][np.concatenate([mine, other])].T).astype(ml_dtypes.bfloat16)
        rmask = (np.zeros((128, 128), np.float32) if h == 0
                 else np.full((128, 128), NEG, np.float32))
        in_maps.append({
            "xT": xTc, "wq": wqt, "wk": wkt, "wv": wvt,
            "dmask": tri, "rmask": rmask,
            "hoff": np.array([[h * 1024, (1 - h) * 1024]], np.uint32),
        })
    return in_maps


def assemble(results):
    z = np.empty((B, T, D), np.float32)
    for c in range(N_CORES):
        b, h = c // 2, c % 2
        o = np.asarray(results[c]["out"]).astype(np.float32)  # [64, 2048]
        for p in range(16):
            g = 2 * (p % 8) + (p // 8)
            z[b, g * 128:(g + 1) * 128, 64 * h:64 * h + 64] = \
                o[:, p * 128:(p + 1) * 128].T
    return z


def kernel(x_in, Wq, Wk, Wv):
    nc = _get_nc()
    in_maps = make_in_maps(x_in, Wq, Wk, Wv)
    res = run_bass_kernel_spmd(nc, in_maps, core_ids=list(range(N_CORES)))
    return assemble(res.results)


# revision 21
# speedup vs baseline: 1.6170x; 1.0433x over previous
"""Trainium2 Bass kernel for nn_AttentionHead (softmax over query axis).

Sharding (8 cores = 4 batches x 2): core pair (2b, 2b+1) handles batch b.
Rank h = c%2 owns KEY blocks of parity h: local chunk sb <-> global key
block gk = 2*sb + h.  Each core projects q for ALL 2048 rows (redundant,
avoids a mid-kernel AllGather) and k/v for its own 1024 rows only.

Per core (single SPMD program; h only appears in host-staged data):
  - host stages xT = x.T in bf16, columns [my 1024 rows | other 1024 rows]
    -> projections need no PE transposes and no gather
  - a tiny dummy collective issued at t~0 absorbs the collective-stream
    init barrier (~21.7us start + ~17us barrier + ~11.5us first-op cost)
    so the real collectives at the end pay only ~1.2us trigger latency
  - scores sT[s, t] = kb.T @ qT for queries t >= key block; causal via
    ADDITIVE masks on psum before exp (diag tri / first-remote-chunk)
  - Z[s] = sum_t E[s, t] is fully local (key-sharded) -> no AllReduce
  - v' = v/Z; AV accumulates zT[d, t] partials over local key blocks into
    4 PSUM banks; the low-column banks finish at sb=3 and ReduceScatter-A
    (bf16, pair, add) ships them while sb=4..7 still compute; RS-B ships
    the rest.  RS splits the D dim: core h ends with z[d in 64h..64h+64].
Host assembles the 8 [64, 2048] bf16 outputs into [4, 2048, 128] f32.
"""
import sys

for _p in ("/opt/trn_rl_repo",):
    if _p not in sys.path:
        sys.path.append(_p)

import numpy as np
import ml_dtypes

import concourse.bass as bass
import concourse.mybir as mybir
import concourse.tile as tile
from concourse import bacc
from concourse.bass import ds, ts
from concourse.bass_utils import run_bass_kernel_spmd
from concourse.masks import make_identity

BF16 = mybir.dt.bfloat16
F32 = mybir.dt.float32
U32 = mybir.dt.uint32
AF = mybir.ActivationFunctionType
ALU = mybir.AluOpType
AX = mybir.AxisListType

B, T, E, D = 4, 2048, 2048, 128
NLC = 8          # local 128-chunks per core (keys); queries = 2*NLC chunks
NE = 16          # E chunks of 128
SCALE = 1.0 / np.sqrt(D)
N_CORES = 8
REPLICA_GROUPS = [[0, 1], [2, 3], [4, 5], [6, 7]]
NEG = -1.0e30


def pieces(sb):
    """Column pieces [c0, width) of the valid query range [sb*128, 1024),
    split at absolute column 512 (PSUM-bank aligned)."""
    lo = sb * 128
    if lo < 512:
        return [(lo, 512 - lo), (512, 512)]
    return [(lo, 1024 - lo)]


def build_nc():
    nc = bacc.Bacc("TRN2", target_bir_lowering=False, debug=False,
                   num_devices=N_CORES)
    xT = nc.dram_tensor("xT", [E, T], BF16, kind="ExternalInput")
    wq = nc.dram_tensor("wq", [128, NE, D], BF16, kind="ExternalInput")
    wk = nc.dram_tensor("wk", [128, NE, D], BF16, kind="ExternalInput")
    wv = nc.dram_tensor("wv", [128, NE, D], BF16, kind="ExternalInput")
    dmask = nc.dram_tensor("dmask", [128, 128], F32, kind="ExternalInput")
    rmask = nc.dram_tensor("rmask", [128, 128], F32, kind="ExternalInput")
    hoff = nc.dram_tensor("hoff", [1, 4], U32, kind="ExternalInput")
    out = nc.dram_tensor("out", [64, T], BF16, kind="ExternalOutput")

    with tile.TileContext(nc) as tc:
        _body(nc, tc, xT, wq, wk, wv, dmask, rmask, hoff, out)
    nc.compile()
    return nc


def _body(nc, tc, xT, wq, wk, wv, dmask, rmask, hoff, out):
    with (
        tc.tile_pool(name="const", bufs=1) as const_pool,
        tc.tile_pool(name="dram", bufs=1, space="DRAM") as dram_pool,
        tc.tile_pool(name="proj", bufs=1) as proj_pool,
        tc.tile_pool(name="escore", bufs=1) as e_pool,
    ):
        # ---- constants / weights (SWDGE: keep HWDGE queues free for x) ----
        ident = const_pool.tile([128, 128], BF16, name="ident")
        make_identity(nc, ident)
        wq_sb = const_pool.tile([128, NE, D], BF16, name="wq_sb")
        wk_sb = const_pool.tile([128, NE, D], BF16, name="wk_sb")
        wv_sb = const_pool.tile([128, NE, D], BF16, name="wv_sb")
        nc.gpsimd.dma_start(out=wk_sb[:], in_=wk[:])
        nc.gpsimd.dma_start(out=wq_sb[:], in_=wq[:])
        nc.gpsimd.dma_start(out=wv_sb[:], in_=wv[:])
        dmask_sb = const_pool.tile([128, 128], F32, name="dmask_sb")
        rmask_sb = const_pool.tile([128, 128], F32, name="rmask_sb")
        nc.gpsimd.dma_start(out=dmask_sb[:], in_=dmask[:])
        nc.gpsimd.dma_start(out=rmask_sb[:], in_=rmask[:])
        stats = const_pool.tile([128, NLC * 4], F32, name="stats")
        nc.vector.memset(stats[:], 0.0)

        # ---- dummy early collective: absorbs cc-stream init + barrier ----
        dummy_sb = const_pool.tile([128, 16], BF16, name="dummy_sb")
        nc.vector.memset(dummy_sb[:], 0.0)
        dummy_in = dram_pool.tile([128, 16], BF16, name="dummy_in")
        dummy_out = dram_pool.tile([64, 16], BF16, name="dummy_out")
        nc.gpsimd.dma_start(out=dummy_in[:], in_=dummy_sb[:])
        nc.gpsimd.collective_compute(
            "ReduceScatter", ALU.add, replica_groups=REPLICA_GROUPS,
            ins=[dummy_in[:].opt()], outs=[dummy_out[:].opt()],
        )

        # runtime pair-rank offsets: hoff = [h, 1-h] * [1024, 512]
        _r = [nc.sync.alloc_register(f"rv{i}") for i in range(4)]
        for i in range(4):
            nc.sync.reg_load(_r[i], hoff[0:1, i:i + 1])
        rv_h = nc.sync.snap(_r[0], donate=True, min_val=0, max_val=1024)
        rv_r = nc.sync.snap(_r[1], donate=True, min_val=0, max_val=1024)
        rv_h5 = nc.sync.snap(_r[2], donate=True, min_val=0, max_val=512)
        rv_r5 = nc.sync.snap(_r[3], donate=True, min_val=0, max_val=512)

        # ---- x load (pre-transposed bf16, fully contiguous, e-ordered) ----
        x_sb = []
        for e in range(NE):
            t_x = const_pool.tile([128, T], BF16, name=f"xT{e}")
            nc.sync.dma_start(out=t_x[:, ds(0, 1024)],
                              in_=xT[ts(e, 128), ds(0, 1024)])
            nc.scalar.dma_start(out=t_x[:, ds(1024, 1024)],
                                in_=xT[ts(e, 128), ds(1024, 1024)])
            x_sb.append(t_x)

        qT_sb = proj_pool.tile([128, T], BF16, name="qT_sb")
        kT_sb = proj_pool.tile([128, NLC * 128], BF16, name="kT_sb")
        vT_sb = proj_pool.tile([128, NLC * 128], BF16, name="vT_sb")
        v_loc = proj_pool.tile([128, NLC * 128], BF16, name="v_loc")
        v_sc = proj_pool.tile([128, NLC * 128], BF16, name="v_sc")
        recip = const_pool.tile([128, NLC], F32, name="recip")

        # ---- k & q projections (k needed first for scores) ----
        with tc.tile_pool(name="pj_psum", bufs=1, space="PSUM") as pj:
            junk = pj.tile([128, 512], F32, tag="k0")
            for i in range(6):
                nc.tensor.matmul(junk[:], lhsT=wk_sb[:, i, :],
                                 rhs=wk_sb[:, ds(0, 4), :],
                                 start=True, stop=True)
            qp = [pj.tile([128, 512], F32, tag=f"p{i}", name=f"qp{i}")
                  for i in range(4)]
            k0 = pj.tile([128, 512], F32, tag="k0")
            k1 = pj.tile([128, 512], F32, tag="k1")
            for e in range(NE):
                st, sp = (e == 0), (e == NE - 1)
                nc.tensor.matmul(k0[:], lhsT=wk_sb[:, e, :],
                                 rhs=x_sb[e][:, ds(0, 512)], start=st, stop=sp)
                nc.tensor.matmul(k1[:], lhsT=wk_sb[:, e, :],
                                 rhs=x_sb[e][:, ds(512, 512)], start=st, stop=sp)
                for i in range(4):
                    nc.tensor.matmul(qp[i][:], lhsT=wq_sb[:, e, :],
                                     rhs=x_sb[e][:, ds(i * 512, 512)],
                                     start=st, stop=sp)
            nc.vector.tensor_copy(out=kT_sb[:, ds(0, 512)], in_=k0[:])
            nc.vector.tensor_copy(out=kT_sb[:, ds(512, 512)], in_=k1[:])
            for i in range(4):
                nc.vector.tensor_copy(out=qT_sb[:, ds(i * 512, 512)], in_=qp[i][:])

        # ---- scores / exp / piece-sums (overlaps the v projection) ----
        with (
            tc.tile_pool(name="vp_psum", bufs=1, space="PSUM") as vp,
            tc.tile_pool(name="sc_psum", bufs=2, space="PSUM") as scp,
            tc.tile_pool(name="zt_psum", bufs=1, space="PSUM") as ztp,
        ):
            e_tiles = []
            for sb in range(NLC):
                esb = e_pool.tile([128, 2048], BF16, name=f"esb{sb}")
                e_tiles.append(esb)
                kb = kT_sb[:, ts(sb, 128)]
                for half in range(2):  # 0: my queries, 1: peer's queries
                    mask = dmask_sb if half == 0 else rmask_sb
                    for pi, (c0, pw) in enumerate(pieces(sb)):
                        sc = scp.tile([128, 512], F32, tag="sc")
                        nc.tensor.matmul(sc[:, ds(0, pw)], lhsT=kb,
                                         rhs=qT_sb[:, ds(half * 1024 + c0, pw)],
                                         start=True, stop=True)
                        if pi == 0:  # chunk on/left of the causal boundary
                            nc.vector.tensor_tensor(
                                out=sc[:, ds(0, 128)], in0=sc[:, ds(0, 128)],
                                in1=mask[:], op=ALU.add)
                        nc.scalar.activation(out=esb[:, ds(half * 1024 + c0, pw)],
                                             in_=sc[:, ds(0, pw)],
                                             func=AF.Exp, scale=SCALE)
                        nc.vector.reduce_sum(
                            out=stats[:, ds(sb * 4 + half * 2 + pi, 1)],
                            in_=esb[:, ds(half * 1024 + c0, pw)], axis=AX.X)

            # ---- v projection + v natural via PE transpose ----
            v0 = vp.tile([128, 512], F32, tag="v0")
            v1 = vp.tile([128, 512], F32, tag="v1")
            for e in range(NE):
                st, sp = (e == 0), (e == NE - 1)
                nc.tensor.matmul(v0[:], lhsT=wv_sb[:, e, :],
                                 rhs=x_sb[e][:, ds(0, 512)], start=st, stop=sp)
                nc.tensor.matmul(v1[:], lhsT=wv_sb[:, e, :],
                                 rhs=x_sb[e][:, ds(512, 512)], start=st, stop=sp)
            nc.vector.tensor_copy(out=vT_sb[:, ds(0, 512)], in_=v0[:])
            nc.vector.tensor_copy(out=vT_sb[:, ds(512, 512)], in_=v1[:])
            for sb in range(NLC):
                tpv = vp.tile([128, 128], BF16, tag="v0")
                nc.tensor.transpose(out=tpv[:], in_=vT_sb[:, ts(sb, 128)],
                                    identity=ident[:])
                nc.vector.tensor_copy(out=v_loc[:, ts(sb, 128)], in_=tpv[:])

            # ---- Z -> v' -> AV, streamed per key block; split RS ----
            zt = [ztp.tile([128, 512], F32, tag=f"zt{j}", name=f"zt{j}")
                  for j in range(4)]
            zT_f = proj_pool.tile([128, 2048], BF16, name="zT_f")
            zinA = dram_pool.tile([128, 1024], BF16, name="zinA")
            zinB = dram_pool.tile([128, 1024], BF16, name="zinB")
            zredA = dram_pool.tile([64, 1024], BF16, name="zredA")
            zredB = dram_pool.tile([64, 1024], BF16, name="zredB")
            for sb in range(NLC):
                esb = e_tiles[sb]
                zs = const_pool.tile([128, 1], F32, tag="zs")
                nc.vector.reduce_sum(out=zs[:], in_=stats[:, ds(sb * 4, 4)],
                                     axis=AX.X)
                nc.vector.reciprocal(out=recip[:, ds(sb, 1)], in_=zs[:])
                nc.vector.tensor_scalar_mul(
                    out=v_sc[:, ts(sb, 128)], in0=v_loc[:, ts(sb, 128)],
                    scalar1=recip[:, ds(sb, 1)])
                vs = v_sc[:, ts(sb, 128)]
                for half in range(2):
                    for (c0, pw) in pieces(sb):
                        j = half * 2 + (c0 // 512)
                        nc.tensor.matmul(
                            zt[j][:, ds(c0 % 512, pw)], lhsT=vs,
                            rhs=esb[:, ds(half * 1024 + c0, pw)],
                            start=(sb == 0),
                            stop=(sb == (3 if c0 < 512 else NLC - 1)),
                            skip_group_check=True)
                if sb == 3:
                    # low-column banks are complete: ship the first RS now
                    nc.vector.tensor_copy(out=zT_f[:, ds(0, 512)], in_=zt[0][:])
                    nc.vector.tensor_copy(out=zT_f[:, ds(1024, 512)],
                                          in_=zt[2][:])
                    nc.sync.dma_start(out=zinA[:, ds(rv_h5, 512)],
                                      in_=zT_f[:, ds(0, 512)])
                    nc.sync.dma_start(out=zinA[:, ds(rv_r5, 512)],
                                      in_=zT_f[:, ds(1024, 512)])
                    nc.gpsimd.collective_compute(
                        "ReduceScatter", ALU.add, replica_groups=REPLICA_GROUPS,
                        ins=[zinA[:].opt()], outs=[zredA[:].opt()],
                    )
                    nc.sync.dma_start(out=out[:, ds(0, 1024)], in_=zredA[:])
            nc.vector.tensor_copy(out=zT_f[:, ds(512, 512)], in_=zt[1][:])
            nc.vector.tensor_copy(out=zT_f[:, ds(1536, 512)], in_=zt[3][:])
            nc.sync.dma_start(out=zinB[:, ds(rv_h5, 512)],
                              in_=zT_f[:, ds(512, 512)])
            nc.sync.dma_start(out=zinB[:, ds(rv_r5, 512)],
                              in_=zT_f[:, ds(1536, 512)])
            nc.gpsimd.collective_compute(
                "ReduceScatter", ALU.add, replica_groups=REPLICA_GROUPS,
                ins=[zinB[:].opt()], outs=[zredB[:].opt()],
            )
            nc.sync.dma_start(out=out[:, ds(1024, 1024)], in_=zredB[:])


_NC_CACHE = None


def _get_nc():
    global _NC_CACHE
    if _NC_CACHE is None:
        _NC_CACHE = build_nc()
    return _NC_CACHE


def _w_tiles(W):
    return np.ascontiguousarray(
        np.asarray(W, np.float32).reshape(NE, 128, D).transpose(1, 0, 2)
    ).astype(ml_dtypes.bfloat16)


def make_in_maps(x_in, Wq, Wk, Wv):
    x_in = np.asarray(x_in, np.float32)
    wqt, wkt, wvt = _w_tiles(Wq), _w_tiles(Wk), _w_tiles(Wv)
    tri = np.where(np.arange(128)[None, :] >= np.arange(128)[:, None],
                   0.0, NEG).astype(np.float32)
    in_maps = []
    for c in range(N_CORES):
        b, h = c // 2, c % 2
        mine = np.concatenate(
            [np.arange((2 * lc + h) * 128, (2 * lc + h + 1) * 128)
             for lc in range(NLC)])
        other = np.concatenate(
            [np.arange((2 * lc + 1 - h) * 128, (2 * lc + 2 - h) * 128)
             for lc in range(NLC)])
        xTc = np.ascontiguousarray(
            x_in[b][np.concatenate([mine, other])].T).astype(ml_dtypes.bfloat16)
        rmask = (np.zeros((128, 128), np.float32) if h == 0
                 else np.full((128, 128), NEG, np.float32))
        in_maps.append({
            "xT": xTc, "wq": wqt, "wk": wkt, "wv": wvt,
            "dmask": tri, "rmask": rmask,
            "hoff": np.array([[h * 1024, (1 - h) * 1024,
                               h * 512, (1 - h) * 512]], np.uint32),
        })
    return in_maps


# out columns -> gpos position: A-half (cols 0..1023) covers positions
# [0-3, 8-11], B-half (cols 1024..2047) covers [4-7, 12-15].
_POS = [0, 1, 2, 3, 8, 9, 10, 11, 4, 5, 6, 7, 12, 13, 14, 15]


def assemble(results):
    z = np.empty((B, T, D), np.float32)
    for c in range(N_CORES):
        b, h = c // 2, c % 2
        o = np.asarray(results[c]["out"]).astype(np.float32)  # [64, 2048]
        for ci in range(16):
            p = _POS[ci]
            g = 2 * (p % 8) + (p // 8)
            z[b, g * 128:(g + 1) * 128, 64 * h:64 * h + 64] = \
                o[:, ci * 128:(ci + 1) * 128].T
    return z


def kernel(x_in, Wq, Wk, Wv):
    nc = _get_nc()
    in_maps = make_in_maps(x_in, Wq, Wk, Wv)
    res = run_bass_kernel_spmd(nc, in_maps, core_ids=list(range(N_CORES)))
    return assemble(res.results)
